# revision 7
# baseline (speedup 1.0000x reference)
"""CriticGNN Trainium2 kernel: 3x GATConv + BN + ReLU + mean-pool + MLP head.

Strategy: edges sharded across 8 cores by dst-node range (12500 nodes/core).
Host pre-sorts edges by (core, dst_tile, src_shard) into 128-edge blocks.
Per layer on device:
  edge phase: dma_gather F[src] rows (512B: h bf16 + al_s/al_d f32), build
    one-hot S0[e,j]=(ldst==iota) on DVE, al_d[dst] per edge via PE transpose
    of S0 + matmul, logits -> lrelu -> exp (ACT), scale messages by ex,
    aggregate per 128-node dst tile with PE matmul S0^T @ [ex*h | ex] into
    PSUM (num + den).  Softmax max-subtraction is skipped (logits are small;
    mathematically identical).
  node phase: num/den, BN (AllReduced stats), ReLU, next-layer h' = X@W and
    al' = X@(W@A) matmuls, pack F' rows, AllGather to replicate F'.
Final: mean-pool partials -> AllReduce -> tiny MLP on every core.
"""
import os
import sys
import types
import numpy as np

N = 100000
E = 1600000
FEAT = 64
HID = 128
NEG = 0.2
EPS = 1e-5
NC = 8
PER = 12500          # nodes per core
TILES = 98           # ceil(12500/128)
SLOTS = TILES * 128  # 12544 padded node slots per core
FSLOTS = NC * SLOTS  # 100352
SHARD = FSLOTS // 4  # 25088 rows per int16-index shard
ROWB16 = 256         # bf16 cols per F row (512 bytes)


def _install_trace_shim():
    try:
        import antenv
        if "antenv.axon_hooks" not in sys.modules:
            mod = types.ModuleType("antenv.axon_hooks")
            mod._hook = None
            mod.set_axon_ntff_profile_hook = lambda h: setattr(mod, "_hook", h)
            mod.get_axon_ntff_profile_hook = lambda: mod._hook
            sys.modules["antenv.axon_hooks"] = mod
            antenv.axon_hooks = mod
        from antenv.axon_hooks import get_axon_ntff_profile_hook, set_axon_ntff_profile_hook
        if get_axon_ntff_profile_hook() is None:
            from trn_agent_boot.trn_boot import _ntff_profile_via_ctypes
            set_axon_ntff_profile_hook(_ntff_profile_via_ctypes('/opt/axon/libaxon_pjrt.so'))
        import concourse.bass_utils as bu
        bu.upload_artifacts = lambda tmpdir: "local://" + str(tmpdir)
    except Exception:
        pass


def _prep_edges(edge_index):
    """Sort/bucket edges; returns (Bts [98,4], per-core IDX16/LDST arrays)."""
    src = np.concatenate([edge_index[0].astype(np.int64), np.arange(N, dtype=np.int64)])
    dst = np.concatenate([edge_index[1].astype(np.int64), np.arange(N, dtype=np.int64)])
    core = dst // PER
    rem = dst % PER
    lt = rem // 128
    ldst = rem % 128
    sslot = (src // PER) * SLOTS + (src % PER)
    shard = sslot // SHARD
    sidx = (sslot % SHARD).astype(np.int64)

    key = ((core * TILES + lt) * 4 + shard).astype(np.int64)
    order = np.argsort(key, kind="stable")
    key_s = key[order]
    cnt = np.bincount(key_s, minlength=NC * TILES * 4).reshape(NC, TILES, 4)
    Bts = -(-cnt.max(axis=0) // 128)          # [98,4] blocks per (tile, shard)
    Bts = np.maximum(Bts, cnt.max(axis=0) > 0)
    Bt = Bts.sum(axis=1)                      # [98]
    TOTB = int(Bt.sum())

    # slot base (in edge-slots) of each (t,s) segment within a core's layout
    seg_base = np.zeros((TILES, 4), np.int64)
    acc = 0
    for t in range(TILES):
        for s in range(4):
            seg_base[t, s] = acc
            acc += Bts[t, s] * 128
    assert acc == TOTB * 128

    # per-edge within-bucket rank
    starts = np.zeros(NC * TILES * 4 + 1, np.int64)
    np.cumsum(cnt.reshape(-1), out=starts[1:])
    rank = np.arange(len(key_s)) - starts[key_s]

    sidx_s = sidx[order]
    ldst_s = ldst[order]
    core_s = core[order]
    t_s = lt[order]
    sh_s = shard[order]

    p = seg_base[t_s, sh_s] + rank            # padded slot position per edge
    gb = p // 128
    pos = p % 128

    IDX16 = np.zeros((NC, 16, TOTB * 8), np.int16)
    LDST = np.full((NC, 128, TOTB), 300.0, np.float32)
    for c in range(NC):
        m = core_s == c
        pc, gbc, posc = p[m], gb[m], pos[m]
        i = pc - seg_base[t_s[m], sh_s[m]] + seg_base[t_s[m], sh_s[m]]  # = pc
        iseg = pc - (seg_base[t_s[m], sh_s[m]])          # within segment
        col16 = (seg_base[t_s[m], sh_s[m]] // 16) + iseg // 16
        IDX16[c, iseg % 16, col16] = sidx_s[m].astype(np.int16)
        LDST[c, posc, gbc] = ldst_s[m]
    IDX16 = np.tile(IDX16, (1, 8, 1))         # replicate to 128 partitions
    return Bts, Bt, TOTB, seg_base, IDX16, LDST


def _amat(a):
    """a [H, C] -> block-diag [H*C, H] f32."""
    H, C = a.shape
    m = np.zeros((H * C, H), np.float32)
    for h in range(H):
        m[h * C:(h + 1) * C, h] = a[h]
    return m


def kernel(**inputs):
    import ml_dtypes
    import concourse.bacc as bacc
    import concourse.mybir as mybir
    import concourse.tile as tile
    from concourse.bass_utils import run_bass_kernel_spmd

    _install_trace_shim()
    f32, bf16, i16 = mybir.dt.float32, mybir.dt.bfloat16, mybir.dt.int16

    x = np.asarray(inputs["x"], np.float32)
    edge_index = np.asarray(inputs["edge_index"])
    Bts, Bt, TOTB, seg_base, IDX16, LDST = _prep_edges(edge_index)
    Bmax = int(Bt.max())

    # ---- host-side constant prep ----
    W = [np.asarray(inputs["W1"], np.float32), np.asarray(inputs["W2"], np.float32),
         np.asarray(inputs["W3"], np.float32)]
    WA = []
    for li, (sn, dn) in enumerate([("as1", "ad1"), ("as2", "ad2"), ("as3", "ad3")]):
        As = _amat(np.asarray(inputs[sn], np.float32))
        Ad = _amat(np.asarray(inputs[dn], np.float32))
        WA.append(W[li] @ np.concatenate([As, Ad], axis=1))      # [K, 2H]
    BN_G = [np.asarray(inputs[k], np.float32).reshape(1, HID) for k in ("g1", "g2", "g3")]
    BN_B = [np.asarray(inputs[k], np.float32).reshape(1, HID) for k in ("be1", "be2", "be3")]
    vW1 = np.asarray(inputs["vW1"], np.float32)
    vb1 = np.asarray(inputs["vb1"], np.float32).reshape(1, HID)
    vW2 = np.asarray(inputs["vW2"], np.float32)
    vb2 = float(np.asarray(inputs["vb2"]).reshape(-1)[0])
    HH = [4, 4, 1]                 # heads per layer
    CC = [32, 32, 128]

    XLOC = np.zeros((NC, SLOTS, FEAT), np.float32)
    for c in range(NC):
        XLOC[c, :PER] = x[c * PER:(c + 1) * PER]

    IOTA_REP = np.tile(np.arange(128, dtype=np.float32), Bmax).reshape(1, -1)
    IOTA_REP = np.broadcast_to(IOTA_REP, (128, Bmax * 128)).astype(ml_dtypes.bfloat16)
    IDF32 = np.eye(128, dtype=np.float32)
    ID16 = np.eye(128, dtype=ml_dtypes.bfloat16)
    ONESC = np.ones((128, 1), np.float32)
    ONESR = np.ones((1, 128), np.float32)

    # ================= bass program =================
    nc = bacc.Bacc("TRN2", target_bir_lowering=False, debug=False, num_devices=NC)
    P_IDX = nc.declare_dram_parameter("IDX16", [128, TOTB * 8], i16, isOutput=False)
    P_LDST = nc.declare_dram_parameter("LDST", [128, TOTB], bf16, isOutput=False)
    P_XLOC = nc.declare_dram_parameter("XLOC", [SLOTS, FEAT], f32, isOutput=False)
    P_IOTA = nc.declare_dram_parameter("IOTA", [128, Bmax * 128], bf16, isOutput=False)
    P_IDF = nc.declare_dram_parameter("IDF", [128, 128], f32, isOutput=False)
    P_ID16 = nc.declare_dram_parameter("ID16", [128, 128], bf16, isOutput=False)
    P_ONESC = nc.declare_dram_parameter("ONESC", [128, 1], f32, isOutput=False)
    P_ONESR = nc.declare_dram_parameter("ONESR", [1, 128], f32, isOutput=False)
    P_W = [nc.declare_dram_parameter(f"W{i+1}", list(W[i].shape), f32, isOutput=False) for i in range(3)]
    P_WA = [nc.declare_dram_parameter(f"WA{i+1}", list(WA[i].shape), f32, isOutput=False) for i in range(3)]
    P_BNG = [nc.declare_dram_parameter(f"BNG{i+1}", [1, HID], f32, isOutput=False) for i in range(3)]
    P_BNB = [nc.declare_dram_parameter(f"BNB{i+1}", [1, HID], f32, isOutput=False) for i in range(3)]
    P_VW1 = nc.declare_dram_parameter("VW1", [HID, HID], f32, isOutput=False)
    P_VB1 = nc.declare_dram_parameter("VB1", [1, HID], f32, isOutput=False)
    P_VW2 = nc.declare_dram_parameter("VW2", [HID, 1], f32, isOutput=False)
    P_OUT = nc.declare_dram_parameter("v_out", [1, 1], f32, isOutput=True)

    FM = [nc.dram_tensor(f"fmine{l}", [SLOTS, ROWB16], bf16) for l in range(3)]
    FF = [nc.dram_tensor(f"ffull{l}", [FSLOTS, ROWB16], bf16, addr_space="Shared") for l in range(3)]
    ST_IN = [nc.dram_tensor(f"stin{l}", [2, HID], f32) for l in range(3)]
    ST_OUT = [nc.dram_tensor(f"stout{l}", [2, HID], f32, addr_space="Shared") for l in range(3)]
    PL_IN = nc.dram_tensor("plin", [1, HID], f32)
    PL_OUT = nc.dram_tensor("plout", [1, HID], f32, addr_space="Shared")

    rg = [list(range(NC))]
    AF = mybir.ActivationFunctionType
    OP = mybir.AluOpType

    with tile.TileContext(nc) as tc:
        with tc.tile_pool(name="const", bufs=1) as cp, \
             tc.tile_pool(name="persist", bufs=1) as pers:
            # constants to SBUF
            IOTA = cp.tile([128, Bmax * 128], bf16); nc.sync.dma_start(out=IOTA[:], in_=P_IOTA[:])
            IDF = cp.tile([128, 128], f32); nc.sync.dma_start(out=IDF[:], in_=P_IDF[:])
            ID16T = cp.tile([128, 128], bf16); nc.sync.dma_start(out=ID16T[:], in_=P_ID16[:])
            ONEC = cp.tile([128, 1], f32); nc.sync.dma_start(out=ONEC[:], in_=P_ONESC[:])
            ONER = cp.tile([1, 128], f32); nc.sync.dma_start(out=ONER[:], in_=P_ONESR[:])
            WS = []
            for i in range(3):
                w = cp.tile(list(W[i].shape), f32); nc.sync.dma_start(out=w[:], in_=P_W[i][:])
                wa = cp.tile(list(WA[i].shape), f32); nc.sync.dma_start(out=wa[:], in_=P_WA[i][:])
                WS.append((w, wa))
            VW1s = cp.tile([HID, HID], f32); nc.sync.dma_start(out=VW1s[:], in_=P_VW1[:])
            VB1s = cp.tile([1, HID], f32); nc.sync.dma_start(out=VB1s[:], in_=P_VB1[:])
            VW2s = cp.tile([HID, 1], f32); nc.sync.dma_start(out=VW2s[:], in_=P_VW2[:])
            BNGs = []
            for i in range(3):
                g = cp.tile([1, HID], f32); nc.sync.dma_start(out=g[:], in_=P_BNG[i][:])
                b = cp.tile([1, HID], f32); nc.sync.dma_start(out=b[:], in_=P_BNB[i][:])
                BNGs.append((g, b))
            IDXS = pers.tile([128, TOTB * 8], i16); nc.sync.dma_start(out=IDXS[:], in_=P_IDX[:])
            LDS = pers.tile([128, TOTB], bf16); nc.sync.dma_start(out=LDS[:], in_=P_LDST[:])
            HOUT = pers.tile([128, SLOTS], f32)   # per-tile cols t*128:(t+1)*128

            # ---------- node phase builder ----------
            def pack_write(tp, pp, t, xT, K, li, fm):
                """xT [K,128j] sbuf -> F rows via W/WA matmuls; write fm rows."""
                w, wa = WS[li]
                h2h = 2 * HH[li]
                hT = pp.tile([128, 128], f32, space="PSUM", tag="hT")
                nc.tensor.matmul(hT[:], w[:], xT, start=True, stop=True)
                a8 = pp.tile([8, 128], f32, space="PSUM", tag="a8")
                nc.tensor.matmul(a8[:h2h, :], wa[:], xT, start=True, stop=True)
                h16 = tp.tile([128, 128], bf16, tag="h16")
                nc.vector.tensor_copy(out=h16[:], in_=hT[:])
                hj = pp.tile([128, 128], bf16, space="PSUM", tag="hj")
                nc.tensor.transpose(hj[:], h16[:], ID16T[:])
                a8s = tp.tile([8, 128], f32, tag="a8s")
                nc.scalar.activation(a8s[:h2h, :], a8[:h2h, :], AF.Copy)
                a8j = pp.tile([128, 8], f32, space="PSUM", tag="a8j")
                nc.tensor.transpose(a8j[:, :h2h], a8s[:h2h, :], IDF[0:h2h, 0:h2h])
                frow = tp.tile([128, ROWB16], bf16, tag="frow")
                nc.vector.tensor_copy(out=frow[:, 0:128], in_=hj[:])
                fal = frow[:].bitcast(f32)  # [128, 128] f32 view
                H = HH[li]
                nc.scalar.activation(
                    fal[:, 64:72].rearrange("p (g o) -> p g o", g=2)[:, :, 0:H],
                    a8j[:, 0:2 * H].rearrange("p (g o) -> p g o", g=2),
                    AF.Copy)
                nc.sync.dma_start(out=fm[t * 128:(t + 1) * 128, :], in_=frow[:])

            # ---------- layer 0: x -> F1 ----------
            with tc.tile_pool(name="n0", bufs=3) as tp, \
                 tc.tile_pool(name="n0p", bufs=1, space="PSUM") as pp:
                for t in range(TILES):
                    xt = tp.tile([128, FEAT], f32, tag="xt")
                    nc.sync.dma_start(out=xt[:], in_=P_XLOC[t * 128:(t + 1) * 128, :])
                    xTp = pp.tile([FEAT, 128], f32, space="PSUM", tag="xT")
                    nc.tensor.transpose(xTp[:], xt[:], IDF[:])
                    xTs = tp.tile([FEAT, 128], f32, tag="xTs")
                    nc.scalar.activation(xTs[:], xTp[:], AF.Copy)
                    pack_write(tp, pp, t, xTs[:], FEAT, 0, FM[0])
            nc.gpsimd.collective_compute("AllGather", OP.bypass, replica_groups=rg,
                                         ins=[FM[0][:]], outs=[FF[0][:]])

            # ---------- layers 1..3 ----------
            for li in range(3):
                H, C = HH[li], CC[li]
                aggc = 128 + H
                with tc.tile_pool(name=f"g{li}", bufs=3) as gpl, \
                     tc.tile_pool(name=f"m{li}", bufs=3) as mpl, \
                     tc.tile_pool(name=f"s{li}", bufs=3) as spl, \
                     tc.tile_pool(name=f"ps{li}", bufs=2, space="PSUM") as ppl, \
                     tc.tile_pool(name=f"pt{li}", bufs=2, space="PSUM") as ptl, \
                     tc.tile_pool(name=f"pa{li}", bufs=2, space="PSUM") as pal:
                    for t in range(TILES):
                        B = int(Bt[t])
                        gboff = int(seg_base[t, 0] // 128)
                        g = gpl.tile([128, Bmax * 128], f32, tag="g")
                        for s in range(4):
                            nb = int(Bts[t, s])
                            if nb == 0:
                                continue
                            ob = int((seg_base[t, s] - seg_base[t, 0]) // 128)
                            nc.gpsimd.dma_gather(
                                out_ap=g[:, ob * 128:(ob + nb) * 128].rearrange(
                                    "p (b e) -> p b e", e=128),
                                in_ap=FF[li][s * SHARD:(s + 1) * SHARD, :].bitcast(f32),
                                idxs_ap=IDXS[:, seg_base[t, s] // 16:(seg_base[t, s] + Bts[t, s] * 128) // 16],
                                num_idxs=nb * 128,
                                num_idxs_reg=nb * 128,
                                elem_size=128)
                        # S0 for whole tile
                        s0 = spl.tile([128, Bmax * 128], bf16, tag="s0")
                        nc.vector.tensor_tensor(
                            out=s0[:, :B * 128],
                            in0=LDS[:, gboff:gboff + B].to_broadcast([128, B, 128]),
                            in1=IOTA[:, :B * 128].rearrange("p (b e) -> p b e", e=128),
                            op=OP.is_equal)
                        # al_d tile [128, H] from FM rows
                        ald = mpl.tile([128, H], f32, tag="ald")
                        nc.sync.dma_start(
                            out=ald[:],
                            in_=FM[li][t * 128:(t + 1) * 128, 136:136 + 2 * H].bitcast(f32))
                        # per-edge al_d via S0 transpose + matmul (4-block batches)
                        alde = pal.tile([128, Bmax * 4], f32, space="PSUM", tag="alde")
                        for b0 in range(0, B, 4):
                            bn_ = min(4, B - b0)
                            s0t = ptl.tile([128, 512], bf16, space="PSUM", tag="s0t")
                            for k in range(bn_):
                                nc.tensor.transpose(
                                    s0t[:, k * 128:(k + 1) * 128],
                                    s0[:, (b0 + k) * 128:(b0 + k + 1) * 128], ID16T[:])
                            s0ts = spl.tile([128, 512], f32, tag="s0ts")
                            nc.scalar.activation(s0ts[:, :bn_ * 128], s0t[:, :bn_ * 128], AF.Copy)
                            for k in range(bn_):
                                nc.tensor.matmul(
                                    alde[:, (b0 + k) * 4:(b0 + k) * 4 + H],
                                    s0ts[:, k * 128:(k + 1) * 128], ald[:],
                                    start=True, stop=True)
                        # logits -> lrelu -> exp, tile level
                        lg = mpl.tile([128, Bmax * 4], f32, tag="lg")
                        nc.vector.tensor_tensor(
                            out=lg[:, :B * 4].rearrange("p (b h) -> p b h", h=4)[:, :, 0:H],
                            in0=g[:].rearrange("p (b e) -> p b e", e=128)[:, 0:B, 64:64 + H],
                            in1=alde[:, :B * 4].rearrange("p (b h) -> p b h", h=4)[:, :, 0:H],
                            op=OP.add)
                        lr = mpl.tile([128, Bmax * 4], f32, tag="lr")
                        nc.scalar.activation(lr[:, :B * 4], lg[:, :B * 4], AF.Lrelu, alpha=NEG)
                        mp = mpl.tile([128, Bmax * 132], bf16, tag="mp")
                        mp3 = mp[:].rearrange("p (b e) -> p b e", e=132)
                        nc.scalar.activation(
                            mp3[:, 0:B, 128:128 + H],
                            lr[:, :B * 4].rearrange("p (b h) -> p b h", h=4)[:, :, 0:H],
                            AF.Exp)
                        # messages = h * ex
                        nc.vector.tensor_tensor(
                            out=mp3[:, 0:B, 0:128],
                            in0=g[:, :B * 128].rearrange("p (b e) -> p b e", e=128)[:, :, 0:64].bitcast(bf16),
                            in1=mp[:].bitcast(bf16).rearrange("p (b e) -> p b e", e=132)[:, 0:B, 128:128 + H].to_broadcast([128, B, H, C]),
                            op=OP.mult)
                        # aggregate
                        agg = ppl.tile([128, aggc], f32, space="PSUM", tag="agg")
                        for b in range(B):
                            nc.tensor.matmul(
                                agg[:], s0[:, b * 128:(b + 1) * 128],
                                mp[:, b * 132:b * 132 + aggc],
                                start=(b == 0), stop=(b == B - 1))
                        # evac: H_OUT = num * 1/(den+eps)
                        den = mpl.tile([128, H], f32, tag="den")
                        nc.scalar.activation(den[:], agg[:, 128:128 + H], AF.Copy, bias=1e-16)
                        rcp = mpl.tile([128, H], f32, tag="rcp")
                        nc.vector.reciprocal(rcp[:], den[:])
                        nc.vector.tensor_tensor(
                            out=HOUT[:, t * 128:(t + 1) * 128].rearrange("p (h c) -> p h c", h=H),
                            in0=agg[:, 0:128].rearrange("p (h c) -> p h c", h=H),
                            in1=rcp[:].to_broadcast([128, H, C]),
                            op=OP.mult)

                # ---- BN stats ----
                with tc.tile_pool(name=f"st{li}", bufs=2) as stp, \
                     tc.tile_pool(name=f"stp{li}", bufs=1, space="PSUM") as spp:
                    s1 = spp.tile([1, HID], f32, space="PSUM", tag="s1")
                    s2 = spp.tile([1, HID], f32, space="PSUM", tag="s2")
                    for t in range(TILES):
                        ht = HOUT[:, t * 128:(t + 1) * 128]
                        sq = stp.tile([128, 128], f32, tag="sq")
                        nc.vector.tensor_tensor(out=sq[:], in0=ht, in1=ht, op=OP.mult)
                        nc.tensor.matmul(s1[:], ONEC[:], ht, start=(t == 0), stop=(t == TILES - 1))
                        nc.tensor.matmul(s2[:], ONEC[:], sq[:], start=(t == 0), stop=(t == TILES - 1))
                    stv = stp.tile([1, HID], f32, tag="stv")
                    nc.scalar.activation(stv[:], s1[:], AF.Copy)
                    stv2 = stp.tile([1, HID], f32, tag="stv2")
                    nc.scalar.activation(stv2[:], s2[:], AF.Copy)
                    nc.sync.dma_start(out=ST_IN[li][0:1, :], in_=stv[:])
                    nc.sync.dma_start(out=ST_IN[li][1:2, :], in_=stv2[:])
                    nc.gpsimd.collective_compute("AllReduce", OP.add, replica_groups=rg,
                                                 ins=[ST_IN[li][:]], outs=[ST_OUT[li][:]])
                    stga = stp.tile([1, HID], f32, tag="stga")
                    nc.sync.dma_start(out=stga[:], in_=ST_OUT[li][0:1, :])
                    stgb = stp.tile([1, HID], f32, tag="stgb")
                    nc.sync.dma_start(out=stgb[:], in_=ST_OUT[li][1:2, :])
                    mu = stp.tile([1, HID], f32, tag="mu")
                    nc.scalar.activation(mu[:], stga[:], AF.Copy, scale=1.0 / N)
                    m2 = stp.tile([1, HID], f32, tag="m2")
                    nc.scalar.activation(m2[:], stgb[:], AF.Copy, scale=1.0 / N)
                    var = stp.tile([1, HID], f32, tag="var")
                    nc.vector.tensor_tensor(out=var[:], in0=mu[:], in1=mu[:], op=OP.mult)
                    nc.vector.tensor_tensor(out=var[:], in0=m2[:], in1=var[:], op=OP.subtract)
                    nc.vector.scalar_tensor_tensor(
                        out=var[:], in0=var[:], scalar=EPS, in1=ONER[:],
                        op0=OP.add, op1=OP.mult)
                    sd = stp.tile([1, HID], f32, tag="sd")
                    nc.scalar.activation(sd[:], var[:], AF.Sqrt)
                    rsd = stp.tile([1, HID], f32, tag="rsd")
                    nc.vector.reciprocal(rsd[:], sd[:])
                    scl = stp.tile([1, HID], f32, tag="scl")
                    nc.vector.tensor_tensor(out=scl[:], in0=rsd[:], in1=BNGs[li][0][:], op=OP.mult)
                    sht = stp.tile([1, HID], f32, tag="sht")
                    nc.vector.tensor_tensor(out=sht[:], in0=mu[:], in1=scl[:], op=OP.mult)
                    nc.vector.tensor_tensor(out=sht[:], in0=BNGs[li][1][:], in1=sht[:], op=OP.subtract)
                    sclp = spp.tile([128, HID], f32, space="PSUM", tag="s1")
                    nc.tensor.matmul(sclp[:], ONER[:], scl[:], start=True, stop=True)
                    shtp = spp.tile([128, HID], f32, space="PSUM", tag="s2")
                    nc.tensor.matmul(shtp[:], ONER[:], sht[:], start=True, stop=True)
                    SCL = stp.tile([128, HID], f32, tag="SCL")
                    nc.scalar.activation(SCL[:], sclp[:], AF.Copy)
                    SHT = stp.tile([128, HID], f32, tag="SHT")
                    nc.scalar.activation(SHT[:], shtp[:], AF.Copy)

                    # ---- node phase: BN -> relu -> next layer / pool ----
                    with tc.tile_pool(name=f"nn{li}", bufs=3) as tp, \
                         tc.tile_pool(name=f"np{li}", bufs=1, space="PSUM") as pp:
                        if li == 2:
                            pl = spp.tile([1, HID], f32, space="PSUM", tag="pl")
                        for t in range(TILES):
                            ht = HOUT[:, t * 128:(t + 1) * 128]
                            xb = tp.tile([128, 128], f32, tag="xb")
                            nc.vector.tensor_tensor(out=xb[:], in0=ht, in1=SCL[:], op=OP.mult)
                            nc.vector.tensor_tensor(out=xb[:], in0=xb[:], in1=SHT[:], op=OP.add)
                            xr = tp.tile([128, 128], f32, tag="xr")
                            nc.scalar.activation(xr[:], xb[:], AF.Relu)
                            if li == 2:
                                nc.tensor.matmul(pl[:], ONEC[:], xr[:],
                                                 start=(t == 0), stop=(t == TILES - 1))
                            else:
                                xTp = pp.tile([128, 128], f32, space="PSUM", tag="xT")
                                nc.tensor.transpose(xTp[:], xr[:], IDF[:])
                                xTs = tp.tile([128, 128], f32, tag="xTs")
                                nc.scalar.activation(xTs[:], xTp[:], AF.Copy)
                                pack_write(tp, pp, t, xTs[:], HID, li + 1, FM[li + 1])
                        if li < 2:
                            nc.gpsimd.collective_compute(
                                "AllGather", OP.bypass, replica_groups=rg,
                                ins=[FM[li + 1][:]], outs=[FF[li + 1][:]])
                        else:
                            pls = stp.tile([1, HID], f32, tag="pls")
                            nc.scalar.activation(pls[:], pl[:], AF.Copy)
                            nc.sync.dma_start(out=PL_IN[:], in_=pls[:])
                            nc.gpsimd.collective_compute(
                                "AllReduce", OP.add, replica_groups=rg,
                                ins=[PL_IN[:]], outs=[PL_OUT[:]])
                            pg = stp.tile([1, HID], f32, tag="pg")
                            nc.sync.dma_start(out=pg[:], in_=PL_OUT[:])
                            # value head
                            gcp = spp.tile([128, 1], f32, space="PSUM", tag="s1")
                            nc.tensor.transpose(gcp[:], pg[:], IDF[0:1, 0:1])
                            gc = stp.tile([128, 1], f32, tag="gc")
                            nc.scalar.activation(gc[:], gcp[:], AF.Copy, scale=1.0 / N)
                            v1p = spp.tile([1, HID], f32, space="PSUM", tag="s2")
                            nc.tensor.matmul(v1p[:], gc[:], VW1s[:], start=True, stop=True)
                            v1 = stp.tile([1, HID], f32, tag="v1")
                            nc.vector.tensor_tensor(out=v1[:], in0=v1p[:], in1=VB1s[:], op=OP.add)
                            v1r = stp.tile([1, HID], f32, tag="v1r")
                            nc.scalar.activation(v1r[:], v1[:], AF.Relu)
                            v1cp = spp.tile([128, 1], f32, space="PSUM", tag="s1")
                            nc.tensor.transpose(v1cp[:], v1r[:], IDF[0:1, 0:1])
                            v1c = stp.tile([128, 1], f32, tag="v1c")
                            nc.scalar.activation(v1c[:], v1cp[:], AF.Copy)
                            v2p = spp.tile([1, 1], f32, space="PSUM", tag="s2")
                            nc.tensor.matmul(v2p[:], v1c[:], VW2s[:], start=True, stop=True)
                            vo = stp.tile([1, 1], f32, tag="vo")
                            nc.scalar.activation(vo[:], v2p[:], AF.Copy, bias=vb2)
                            nc.sync.dma_start(out=P_OUT[:], in_=vo[:])

    nc.compile()

    def bft(a):
        return np.ascontiguousarray(a).astype(ml_dtypes.bfloat16)

    base = {
        "IOTA": np.ascontiguousarray(IOTA_REP), "IDF": IDF32, "ID16": ID16,
        "ONESC": ONESC, "ONESR": ONESR, "VW1": vW1, "VB1": vb1, "VW2": vW2,
    }
    for i in range(3):
        base[f"W{i+1}"] = W[i]
        base[f"WA{i+1}"] = WA[i]
        base[f"BNG{i+1}"] = BN_G[i]
        base[f"BNB{i+1}"] = BN_B[i]
    in_maps = []
    for c in range(NC):
        m = dict(base)
        m["IDX16"] = IDX16[c]
        m["LDST"] = bft(LDST[c])
        m["XLOC"] = XLOC[c]
        in_maps.append(m)

    trace = bool(os.environ.get("BASS_TRACE"))
    res = run_bass_kernel_spmd(nc, in_maps, list(range(NC)), trace=trace)
    if trace and res.exec_time_ns is not None:
        print(f"HW exec time: {res.exec_time_ns} ns")
    return np.asarray(res.results[0]["v_out"], np.float32)
